# revision 2
# baseline (speedup 1.0000x reference)
"""GraphConv (dgl norm='both') distributed Bass kernel for 8 trn2 NeuronCores.

out = relu( D_in^{-1/2} A D_out^{-1/2} (h W) + b )

Sharding: nodes are range-partitioned across the 8 cores (12500 each, padded
to 12544 = 98*128). Each core:
  phase 1: counts out-degrees of its own nodes from the src-bucketed edge
           list (selection-matrix matmuls against a ones vector),
  phase 2: projects x = (h_shard * norm_src) @ W and appends a ones column,
  phase 3: AllGather of x -> x_full (all 8 shards, in DRAM),
  phase 4: for its dst-bucketed edges: indirect-DMA row gathers of x_full,
           one-hot selection matmuls accumulating [32-node-block, 65] tiles
           in PSUM (the ones column yields the in-degree for free),
  phase 5: per block: norm = rsqrt(max(deg,1)), out = relu(agg * norm) (+b).

Edges are bucketed on the host by (owning core, 32-node cell) with a shared
per-cell chunk-count profile (max across cores) so that all 8 cores run one
identical SPMD program; pad slots use gather row 0 and compare value -1
(whose selection column is all-zero, so they contribute nothing).
"""

import sys

if "/opt/trn_rl_repo" not in sys.path:
    sys.path.insert(0, "/opt/trn_rl_repo")

import numpy as np
from contextlib import ExitStack

import concourse.bass as bass
import concourse.bacc as bacc
import concourse.mybir as mybir
import concourse.tile as tile
from concourse import bass_utils

P = 128
NCORES = 8
N_NODES = 100000
IN_DIM = 256
OUT_DIM = 64
D = OUT_DIM + 1          # features + ones column
NLOC = N_NODES // NCORES  # 12500
GROUPS = 98               # 128-node groups per core
NPAD = GROUPS * P         # 12544
CELL = 32
NCELLS = NPAD // CELL     # 392
NFULL = NCORES * NPAD
SBATCH = 16               # chunks per batched selection-matrix build

F32 = mybir.dt.float32
I32 = mybir.dt.int32


def _bucket(values_cmp, cells, gidx, ncells):
    """Per-core bucketing: returns (counts, order) with edges sorted by cell."""
    order = np.argsort(cells, kind="stable")
    counts = np.bincount(cells, minlength=ncells)
    return counts, order


def _fill(buf_cmp, buf_gidx, cells_sorted, cmp_sorted, gidx_sorted, counts, off):
    starts = np.concatenate([[0], np.cumsum(counts)[:-1]])
    rank = np.arange(cells_sorted.shape[0]) - starts[cells_sorted]
    pos = off[cells_sorted] + rank
    buf_cmp[pos] = cmp_sorted
    if buf_gidx is not None:
        buf_gidx[pos] = gidx_sorted


def prepare_inputs(h, src, dst, W, b):
    """Host-side sharding / marshalling (layout only, no graph math)."""
    src = np.asarray(src).astype(np.int64)
    dst = np.asarray(dst).astype(np.int64)
    h = np.asarray(h, dtype=np.float32)
    W = np.asarray(W, dtype=np.float32)
    b = np.asarray(b, dtype=np.float32)

    owner_s = src // NLOC
    s_loc = (src - owner_s * NLOC).astype(np.int64)
    grow = (owner_s * NPAD + s_loc).astype(np.int32)  # row in padded x_full
    owner_d = dst // NLOC
    d_loc = (dst - owner_d * NLOC).astype(np.int64)

    # ---- main pass: bucket by (dst owner, dst 32-cell) ----
    m_counts = np.zeros((NCORES, NCELLS), np.int64)
    m_data = []
    for k in range(NCORES):
        m = owner_d == k
        dl = d_loc[m]
        cells = (dl // CELL).astype(np.int64)
        cmpv = (dl - cells * CELL).astype(np.float32)
        gi = grow[m]
        order = np.argsort(cells, kind="stable")
        m_counts[k] = np.bincount(cells, minlength=NCELLS)
        m_data.append((cells[order], cmpv[order], gi[order]))
    nch = np.ceil(m_counts / P).max(axis=0).astype(np.int64)  # shared profile
    m_off = np.concatenate([[0], np.cumsum(nch)]) * P
    tc_main = int(m_off[-1]) // P

    main_gidx = np.zeros((NCORES, tc_main * P), np.int32)
    main_cmp = np.full((NCORES, tc_main * P), -1.0, np.float32)
    for k in range(NCORES):
        cells_s, cmp_s, gi_s = m_data[k]
        _fill(main_cmp[k], main_gidx[k], cells_s, cmp_s, gi_s, m_counts[k], m_off[:-1])

    # ---- degree pre-pass: bucket src-locals by (src owner, src 32-cell) ----
    p_counts = np.zeros((NCORES, NCELLS), np.int64)
    p_data = []
    for k in range(NCORES):
        m = owner_s == k
        sl = s_loc[m]
        cells = (sl // CELL).astype(np.int64)
        cmpv = (sl - cells * CELL).astype(np.float32)
        order = np.argsort(cells, kind="stable")
        p_counts[k] = np.bincount(cells, minlength=NCELLS)
        p_data.append((cells[order], cmpv[order]))
    mch = np.ceil(p_counts / P).max(axis=0).astype(np.int64)
    p_off = np.concatenate([[0], np.cumsum(mch)]) * P
    tc_pre = int(p_off[-1]) // P

    pre_cmp = np.full((NCORES, tc_pre * P), -1.0, np.float32)
    for k in range(NCORES):
        cells_s, cmp_s = p_data[k]
        _fill(pre_cmp[k], None, cells_s, cmp_s, None, p_counts[k], p_off[:-1])

    # ---- per-core tensors ----
    hT = np.zeros((NCORES, IN_DIM, NPAD), np.float32)
    for k in range(NCORES):
        hT[k, :, :NLOC] = h[k * NLOC : (k + 1) * NLOC].T
    iota = np.tile(np.arange(CELL, dtype=np.float32), SBATCH)
    iota_rep = np.broadcast_to(iota, (P, SBATCH * CELL)).copy()
    b_rep = np.broadcast_to(b, (P, OUT_DIM)).copy()

    in_maps = []
    for k in range(NCORES):
        in_maps.append(
            {
                "hT_in": np.ascontiguousarray(hT[k]),
                "W_in": W,
                "brep_in": b_rep,
                "iota_in": iota_rep,
                "mgidx_in": np.ascontiguousarray(
                    main_gidx[k].reshape(tc_main, P).T
                ),
                "mcmp_in": np.ascontiguousarray(main_cmp[k].reshape(tc_main, P).T),
                "pcmp_in": np.ascontiguousarray(pre_cmp[k].reshape(tc_pre, P).T),
            }
        )
    return in_maps, nch, mch, tc_main, tc_pre, bool(np.any(b != 0.0))


def build_program(nch, mch, tc_main, tc_pre, has_bias):
    nc = bacc.Bacc(
        "TRN2", target_bir_lowering=False, debug=False, num_devices=NCORES
    )

    hT_in = nc.dram_tensor("hT_in", [IN_DIM, NPAD], F32, kind="ExternalInput")
    W_in = nc.dram_tensor("W_in", [IN_DIM, OUT_DIM], F32, kind="ExternalInput")
    brep_in = nc.dram_tensor("brep_in", [P, OUT_DIM], F32, kind="ExternalInput")
    iota_in = nc.dram_tensor("iota_in", [P, SBATCH * CELL], F32, kind="ExternalInput")
    mgidx_in = nc.dram_tensor("mgidx_in", [P, tc_main], I32, kind="ExternalInput")
    mcmp_in = nc.dram_tensor("mcmp_in", [P, tc_main], F32, kind="ExternalInput")
    pcmp_in = nc.dram_tensor("pcmp_in", [P, tc_pre], F32, kind="ExternalInput")
    out_dram = nc.dram_tensor("out", [NPAD, OUT_DIM], F32, kind="ExternalOutput")

    x_loc = nc.dram_tensor("x_loc", [NPAD, D], F32)
    x_full = nc.dram_tensor("x_full", [NFULL, D], F32, addr_space="Shared")

    with ExitStack() as ctx:
        tc = ctx.enter_context(tile.TileContext(nc))
        const = ctx.enter_context(tc.tile_pool(name="const", bufs=1))

        # persistent tiles
        iota_t = const.tile([P, SBATCH * CELL], F32, tag="iota")
        W0 = const.tile([P, OUT_DIM], F32, tag="W0")
        W1 = const.tile([P, OUT_DIM], F32, tag="W1")
        ones_t = const.tile([P, 1], F32, tag="ones")
        normsrc = const.tile([P, GROUPS], F32, tag="normsrc")
        pcmp_t = const.tile([P, tc_pre], F32, tag="pcmp")
        mcmp_t = const.tile([P, tc_main], F32, tag="mcmp")
        mgidx_t = const.tile([P, tc_main], I32, tag="mgidx")
        brep_t = const.tile([P, OUT_DIM], F32, tag="brep")

        nc.sync.dma_start(out=iota_t[:], in_=iota_in[:, :])
        nc.sync.dma_start(out=W0[:], in_=W_in[0:P, :])
        nc.sync.dma_start(out=W1[:], in_=W_in[P : 2 * P, :])
        nc.sync.dma_start(out=pcmp_t[:], in_=pcmp_in[:, :])
        nc.sync.dma_start(out=mcmp_t[:], in_=mcmp_in[:, :])
        nc.sync.dma_start(out=mgidx_t[:], in_=mgidx_in[:, :])
        nc.sync.dma_start(out=brep_t[:], in_=brep_in[:, :])
        nc.vector.memset(ones_t[:], 1.0)

        # ---------------- phase 1: out-degree pre-pass ----------------
        with (
            tc.tile_pool(name="pre_sb", bufs=4) as pre_sb,
            tc.tile_pool(name="pre_ps", bufs=4, space="PSUM") as pre_ps,
        ):
            j = 0
            Sw = None
            for g in range(GROUPS):
                deg4 = pre_sb.tile([P, 1], F32, tag="deg4")
                for sub in range(4):
                    cell = g * 4 + sub
                    nchunks = int(mch[cell])
                    dps = pre_ps.tile([CELL, 1], F32, space="PSUM", tag="dps")
                    if nchunks == 0:
                        nc.vector.memset(dps[:], 0.0)
                    for c in range(nchunks):
                        if j % SBATCH == 0:
                            w = min(SBATCH, tc_pre - j)
                            Sw = pre_sb.tile([P, SBATCH * CELL], F32, tag="Spre")
                            nc.vector.tensor_tensor(
                                out=Sw[:, : w * CELL],
                                in0=pcmp_t[:, j : j + w].to_broadcast([P, w, CELL]),
                                in1=iota_t[:, : w * CELL],
                                op=mybir.AluOpType.is_equal,
                            )
                        jj = j % SBATCH
                        nc.tensor.matmul(
                            out=dps[:],
                            lhsT=Sw[:, jj * CELL : (jj + 1) * CELL],
                            rhs=ones_t[:],
                            start=(c == 0),
                            stop=(c == nchunks - 1),
                        )
                        j += 1
                    nc.vector.tensor_scalar_max(
                        deg4[CELL * sub : CELL * (sub + 1), :], dps[:], 1.0
                    )
                rcp = pre_sb.tile([P, 1], F32, tag="rcp")
                nc.vector.reciprocal(rcp[:], deg4[:])
                nc.scalar.sqrt(normsrc[:, g : g + 1], rcp[:])

        # ---------------- phase 2: x = (h * norm_src) @ W, ones col ----------------
        with (
            tc.tile_pool(name="xb_sb", bufs=4) as xb_sb,
            tc.tile_pool(name="xb_ps", bufs=2, space="PSUM") as xb_ps,
        ):
            for g in range(GROUPS):
                hta = xb_sb.tile([P, P], F32, tag="hta")
                htb = xb_sb.tile([P, P], F32, tag="htb")
                nc.sync.dma_start(out=hta[:], in_=hT_in[0:P, g * P : (g + 1) * P])
                nc.sync.dma_start(
                    out=htb[:], in_=hT_in[P : 2 * P, g * P : (g + 1) * P]
                )
                xps = xb_ps.tile([P, OUT_DIM], F32, space="PSUM", tag="xps")
                nc.tensor.matmul(out=xps[:], lhsT=hta[:], rhs=W0[:], start=True, stop=False)
                nc.tensor.matmul(out=xps[:], lhsT=htb[:], rhs=W1[:], start=False, stop=True)
                xsb = xb_sb.tile([P, D], F32, tag="xsb")
                nc.vector.tensor_scalar(
                    out=xsb[:, :OUT_DIM],
                    in0=xps[:],
                    scalar1=normsrc[:, g : g + 1],
                    scalar2=None,
                    op0=mybir.AluOpType.mult,
                )
                nc.vector.memset(xsb[:, OUT_DIM:D], 1.0)
                nc.sync.dma_start(
                    out=x_loc[g * P : (g + 1) * P, :], in_=xsb[:]
                )

        # ---------------- phase 3: AllGather ----------------
        nc.gpsimd.collective_compute(
            "AllGather",
            mybir.AluOpType.bypass,
            replica_groups=[list(range(NCORES))],
            ins=[x_loc.ap().opt()],
            outs=[x_full.ap().opt()],
        )

        # ---------------- phase 4+5: gather, scatter matmuls, epilogue ----------------
        with (
            tc.tile_pool(name="mn_sb", bufs=4) as mn_sb,
            tc.tile_pool(name="mn_msg", bufs=12) as mn_msg,
            tc.tile_pool(name="mn_ps", bufs=6, space="PSUM") as mn_ps,
        ):
            j = 0
            Sw = None
            for g in range(GROUPS):
                ost = mn_sb.tile([P, OUT_DIM], F32, tag="ost")
                deg4 = mn_sb.tile([P, 1], F32, tag="deg4m")
                accs = []
                for sub in range(4):
                    cell = g * 4 + sub
                    nchunks = int(nch[cell])
                    acc = mn_ps.tile([CELL, D], F32, space="PSUM", tag="acc")
                    accs.append(acc)
                    if nchunks == 0:
                        nc.vector.memset(acc[:], 0.0)
                    for c in range(nchunks):
                        if j % SBATCH == 0:
                            w = min(SBATCH, tc_main - j)
                            Sw = mn_sb.tile([P, SBATCH * CELL], F32, tag="Smain")
                            nc.vector.tensor_tensor(
                                out=Sw[:, : w * CELL],
                                in0=mcmp_t[:, j : j + w].to_broadcast([P, w, CELL]),
                                in1=iota_t[:, : w * CELL],
                                op=mybir.AluOpType.is_equal,
                            )
                        jj = j % SBATCH
                        msg = mn_msg.tile([P, D], F32, tag="msg")
                        nc.gpsimd.indirect_dma_start(
                            out=msg[:],
                            out_offset=None,
                            in_=x_full[:, :],
                            in_offset=bass.IndirectOffsetOnAxis(
                                ap=mgidx_t[:, j : j + 1], axis=0
                            ),
                        )
                        nc.tensor.matmul(
                            out=acc[:],
                            lhsT=Sw[:, jj * CELL : (jj + 1) * CELL],
                            rhs=msg[:],
                            start=(c == 0),
                            stop=(c == nchunks - 1),
                        )
                        j += 1
                    nc.vector.tensor_scalar_max(
                        deg4[CELL * sub : CELL * (sub + 1), :],
                        acc[:, OUT_DIM:D],
                        1.0,
                    )
                rcp = mn_sb.tile([P, 1], F32, tag="rcpm")
                norm4 = mn_sb.tile([P, 1], F32, tag="norm4")
                nc.vector.reciprocal(rcp[:], deg4[:])
                nc.scalar.sqrt(norm4[:], rcp[:])
                for sub in range(4):
                    sl = slice(CELL * sub, CELL * (sub + 1))
                    if has_bias:
                        nc.vector.tensor_scalar(
                            out=ost[sl, :],
                            in0=accs[sub][:, :OUT_DIM],
                            scalar1=norm4[sl, :],
                            scalar2=None,
                            op0=mybir.AluOpType.mult,
                        )
                        nc.vector.tensor_tensor(
                            out=ost[sl, :],
                            in0=ost[sl, :],
                            in1=brep_t[sl, :],
                            op=mybir.AluOpType.add,
                        )
                        nc.scalar.activation(
                            ost[sl, :],
                            ost[sl, :],
                            mybir.ActivationFunctionType.Relu,
                        )
                    else:
                        nc.scalar.activation(
                            ost[sl, :],
                            accs[sub][:, :OUT_DIM],
                            mybir.ActivationFunctionType.Relu,
                            scale=norm4[sl, :],
                        )
                nc.sync.dma_start(
                    out=out_dram[g * P : (g + 1) * P, :], in_=ost[:]
                )

    nc.compile()
    return nc


def kernel(h, src, dst, W, b):
    in_maps, nch, mch, tc_main, tc_pre, has_bias = prepare_inputs(h, src, dst, W, b)
    nc = build_program(nch, mch, tc_main, tc_pre, has_bias)
    res = bass_utils.run_bass_kernel_spmd(
        nc, in_maps, core_ids=list(range(NCORES))
    )
    out = np.concatenate(
        [res.results[k]["out"][:NLOC] for k in range(NCORES)], axis=0
    )
    return out.astype(np.float32)
